# revision 27
# baseline (speedup 1.0000x reference)
"""Trainium2 Bass kernel for nn_NewSepConv (per-pixel separable conv, K=17).

out[b,c,h,w] = sum_{u,v} pad[b,c,h+u,w+v] * vers[b,u,h,w] * hors[b,v,h,w]
where pad = replication-pad(imgs, 8).

Decomposition (per batch b, output column w):
  E[h, c, v] = sum_r pad[b, c, r, w+v] * C_w[r, h]          (TensorE matmul)
      with band matrix C_w[r, h] = vers[b, r-h, h, w] (0 <= r-h < 17)
  out[b, c, h, w] = sum_v hors[b, v, h, w] * E[h, c, v]      (DVE/Pool mult,
                                                              DVE tree reduce)

v2 vs the 32.3us baseline:
  - The band matrix is built ON-CHIP by a skewed (diagonal) DMA: compact
    vers [h, u, bi, w] streams from HBM straight into the band positions
    band[h+u, h] of a persistent SBUF tile whose off-band area is zeroed
    once at startup.  This cuts band DMA traffic from 5.24MB (expanded,
    77% zeros) to 1.11MB per core.
  - Output ships bf16 (host converts), halving the out DMA.
  - Stage-2 rebalanced: Act does the PSUM->bf16 copies, DVE does a 5-level
    tensor_tensor add tree for the v-reduce (750ns vs 910ns tensor_reduce)
    plus some multiplies, GPSIMD takes the other multiplies via
    scalar_tensor_tensor (1228ns vs tensor_tensor's 1714ns).

Sharding: 8 cores, each takes a 32-column w-chunk (all batches, all rows).
"""

import os
import numpy as np

import concourse.mybir as mybir
import concourse.tile as tile
from concourse import bacc
from concourse.ap import AP
from concourse.bass_utils import run_bass_kernel_spmd

F32 = mybir.dt.float32
BF16 = mybir.dt.bfloat16
NPBF16 = mybir.dt.np(mybir.dt.bfloat16)

B, C, H, W = 4, 3, 256, 256
K = 17
PAD = 8
NCORES = 8
WCHUNK = W // NCORES  # 32

T = 64                # h-tile size
NT = H // T           # 4 h-tiles
KT = T + K - 1        # 80 r-rows per tile
NP_ = 2               # psum pairs (2 h-tiles each -> 128 partitions)
WG = 16               # w-columns per psum tile (2-bank padded slots)
NWG = WCHUNK // WG    # 2

# stage-2 multiply engine per unit index i = 8*g + 4*bi + 2*p + wg (16 units):
# GPSIMD tensor_tensor for these, DVE for the rest.
_POOL_MULT = {1, 5, 9, 13}

import os as _os
_HINTS = [float(x) for x in _os.environ.get(
    "SEPCONV_HINTS", "0.004,0.002,0.008,0.009,0.010").split(",")]

_CACHE = {}


def _build_nc():
    import os
    stage2 = os.environ.get("SEPCONV_STAGE2", "tree")  # tree | reduce
    psrow = os.environ.get("SEPCONV_PSROW", "0") == "1"
    hostband = os.environ.get("SEPCONV_HOSTBAND", "0") == "1"
    l45pool = os.environ.get("SEPCONV_L45", "dve") == "pool"
    pool_mult = set(int(x) for x in os.environ.get(
        "SEPCONV_POOLMULT", "0,2,4,6,8,10,12,14").split(",") if x != "")

    nc = bacc.Bacc("TRN2", target_bir_lowering=False, debug=False)
    padk = nc.dram_tensor("padk", [B, KT, NT, C, WCHUNK + K - 1], BF16,
                          kind="ExternalInput").ap()
    # compact vers, laid out for the skewed band DMA: [g, t, h, u, bi, w]
    versk = nc.dram_tensor("versk", [B // 2, NT, T, K, 2, WCHUNK], BF16,
                           kind="ExternalInput").ap()
    bandk = None
    if hostband:
        bandk = nc.dram_tensor("bandk", [B // 2, NT, KT, T, 2, WCHUNK], BF16,
                               kind="ExternalInput").ap()
    horsk = nc.dram_tensor("horsk", [B, 128, NP_, WCHUNK, K], BF16,
                           kind="ExternalInput").ap()
    outk = nc.dram_tensor("outk", [B // 2, 128, NP_, 2, WCHUNK, C], BF16,
                          kind="ExternalOutput").ap()

    with tile.TileContext(nc) as tc:
        with tc.tile_pool(name="bands", bufs=1) as band_pool, \
             tc.tile_pool(name="pads", bufs=2) as pad_pool, \
             tc.tile_pool(name="hors", bufs=2) as hors_pool, \
             tc.tile_pool(name="mtmp", bufs=12) as m_pool, \
             tc.tile_pool(name="tree", bufs=12) as t_pool, \
             tc.tile_pool(name="obuf", bufs=2) as o_pool, \
             tc.tile_pool(name="psum", bufs=4, space="PSUM") as psum_pool:
            NBAND = 4
            bands = [band_pool.tile([KT, T, 2, WCHUNK], BF16, tag=f"band{q}",
                                    name=f"band{q}")
                     for q in range(NBAND)]
            FREE = T * 2 * WCHUNK  # flat per-partition elements (4096)

            # PE pstate warmup: the cost model ramps the PE clock from
            # 0.65GHz to 2.4GHz over ~3us after the first matmul; two dummy
            # matmuls at t~0.2us start that clock early.
            warm = m_pool.tile([128, 64], BF16, tag="warm", name="warm")
            nc.vector._memset_packed(warm[:].bitcast(mybir.dt.uint32), 0)
            psw = psum_pool.tile([128, 2 * WG, 64] if psrow
                                 else [128, WG, 64], F32, tag="ps",
                                 name="psw", bufs=2 if psrow else 4)
            for _ in range(2):
                nc.tensor.matmul(out=psw[0:2, 0, 0:64], lhsT=warm[:, 0:2],
                                 rhs=warm[:], start=True, stop=True)

            # one-time zeroing of the band rects (uint32 view halves the
            # cost); buffer 0 first (it gates the first skew DMA), split
            # across DVE and GPSIMD.
            f0 = bands[0][:].rearrange("p a b c -> p (a b c)").bitcast(
                mybir.dt.uint32)
            half = FREE // 4
            nc.vector._memset_packed(f0[:, 0:half], 0)
            nc.gpsimd._memset_packed(f0[:, half:2 * half], 0)
            for q in range(1, NBAND):
                flat = bands[q][:].rearrange("p a b c -> p (a b c)")
                eng = nc.gpsimd if q % 2 == 1 else nc.vector
                eng._memset_packed(flat.bitcast(mybir.dt.uint32), 0)

            def skew_dma(g, t, q):
                if hostband:
                    nc.sync.dma_start(out=bands[q][:], in_=bandk[g, t])
                    return
                # band[h+u, h, bi, w] = versk[g, t, h, u, bi, w]
                dst = AP(bands[q][:].tensor, 0,
                         [[FREE + 2 * WCHUNK, T], [FREE, K],
                          [1, 2 * WCHUNK]])
                src = AP(versk.tensor,
                         versk.offset + (g * NT + t) * (T * K * 2 * WCHUNK),
                         [[K * 2 * WCHUNK, T], [2 * WCHUNK, K],
                          [1, 2 * WCHUNK]])
                nc.sync.dma_start(out=dst, in_=src)

            def _emit_reduce_tree(mt, out_ap, last):
                # add-tree over v=17; L1-L3 on DVE (2x bf16), L4-L5 on
                # GPSIMD (DVE for the final row to keep the tail off the
                # Pool queue).
                sh = list(mt.shape[1:-1])
                eng45 = nc.vector if (last or not l45pool) else nc.gpsimd
                a = t_pool.tile([128] + sh + [8], BF16, tag="ta", name="ta")
                nc.vector.tensor_tensor(out=a[:], in0=mt[..., 0:8],
                                        in1=mt[..., 8:16],
                                        op=mybir.AluOpType.add)
                b2 = t_pool.tile([128] + sh + [4], BF16, tag="tb", name="tb")
                nc.vector.tensor_tensor(out=b2[:], in0=a[..., 0:4],
                                        in1=a[..., 4:8],
                                        op=mybir.AluOpType.add)
                c2 = t_pool.tile([128] + sh + [2], BF16, tag="tc", name="tc")
                nc.vector.tensor_tensor(out=c2[:], in0=b2[..., 0:2],
                                        in1=b2[..., 2:4],
                                        op=mybir.AluOpType.add)
                d = t_pool.tile([128] + sh, BF16, tag="td", name="td")
                eng45.tensor_tensor(out=d[:], in0=c2[..., 0],
                                    in1=c2[..., 1],
                                    op=mybir.AluOpType.add)
                eng45.tensor_tensor(out=out_ap, in0=d[:], in1=mt[..., 16],
                                    op=mybir.AluOpType.add)

            for g in range(B // 2):
                pad_all = pad_pool.tile([KT, 2, NT, C, WCHUNK + K - 1], BF16,
                                        tag="pad")
                with tc.tile_wait_until(_HINTS[2], enable=(g > 0)):
                    nc.sync.dma_start(out=pad_all[:, 0], in_=padk[2 * g])
                if g == 0:
                    skew_dma(0, 0, 0)
                    skew_dma(0, 1, 1)
                with tc.tile_wait_until(_HINTS[1] if g == 0 else _HINTS[3]):
                    nc.sync.dma_start(out=pad_all[:, 1], in_=padk[2 * g + 1])
                hors_all = hors_pool.tile([128, 2, NP_, WCHUNK, K], BF16,
                                          tag="hors")
                with tc.tile_wait_until(_HINTS[0] if g == 0 else _HINTS[4]):
                    nc.scalar.dma_start(
                        out=hors_all[:],
                        in_=horsk[2 * g:2 * g + 2].transpose([1, 0, 2, 3, 4]))
                if g == 0:
                    skew_dma(0, 2, 2)
                    skew_dma(0, 3, 3)
                ob = o_pool.tile([128, NP_, 2, WCHUNK, C], BF16, tag="ob")
                for p in range(NP_):
                    for bi in range(2):
                        if psrow:
                            ps = psum_pool.tile([128, 2 * WG, 64], F32,
                                                tag="ps", name="ps", bufs=2)
                            for half in range(2):
                                t = 2 * p + half
                                for wl in range(2 * WG):
                                    nc.tensor.matmul(
                                        out=ps[64 * half:64 * half + 64,
                                               wl, 0:C * K],
                                        lhsT=bands[(2 * (g * NP_ + p)
                                                     + half) % NBAND]
                                            [:, :, bi, wl],
                                        rhs=pad_all[:, bi, t, :, wl:wl + K],
                                        start=True, stop=True)
                            et = m_pool.tile([128, 2 * WG, C, K], BF16,
                                             tag="et", name="et", bufs=4)
                            nc.scalar.copy(
                                out=et[:],
                                in_=ps[:, :, 0:C * K].rearrange(
                                    "p w (c v) -> p w c v", c=C))
                            ets = [et[:, 0:WG], et[:, WG:2 * WG]]
                        else:
                            ets = []
                            for wg in range(NWG):
                                ps = psum_pool.tile([128, WG, 64], F32,
                                                    tag="ps", name="ps",
                                                    bufs=4)
                                for half in range(2):
                                    t = 2 * p + half
                                    for wl8 in range(WG):
                                        wl = wg * WG + wl8
                                        nc.tensor.matmul(
                                            out=ps[64 * half:64 * half + 64,
                                                   wl8, 0:C * K],
                                            lhsT=bands[(2 * (g * NP_ + p)
                                                         + half) % NBAND]
                                                [:, :, bi, wl],
                                            rhs=pad_all[:, bi, t, :,
                                                        wl:wl + K],
                                            start=True, stop=True)
                                et = m_pool.tile([128, WG, C, K], BF16,
                                                 tag="et", name="et", bufs=8)
                                nc.scalar.copy(
                                    out=et[:],
                                    in_=ps[:, :, 0:C * K].rearrange(
                                        "p w (c v) -> p w c v", c=C))
                                ets.append(et)
                        mt = m_pool.tile([128, 2 * WG, C, K], BF16, tag="mt",
                                         name="mt", bufs=4)
                        for wg in range(NWG):
                            i = 8 * g + 4 * bi + 2 * p + wg
                            hslice = hors_all[:, bi, p,
                                              wg * WG:(wg + 1) * WG, :]
                            hb = hslice.unsqueeze(2).broadcast_to(
                                [128, WG, C, K])
                            eng = nc.gpsimd if i in pool_mult else nc.vector
                            eng.tensor_tensor(
                                out=mt[:, wg * WG:(wg + 1) * WG],
                                in0=ets[wg][:] if not psrow else ets[wg],
                                in1=hb, op=mybir.AluOpType.mult)
                        last = (g == B // 2 - 1 and p == NP_ - 1 and bi == 1)
                        if stage2 == "tree":
                            _emit_reduce_tree(mt[:], ob[:, p, bi, :, :],
                                              last)
                        else:
                            with nc.allow_low_precision(
                                    reason="17-term v-sum in bf16 is within "
                                           "the 2e-2 tolerance"):
                                nc.vector.tensor_reduce(
                                    out=ob[:, p, bi, :, :], in_=mt[:],
                                    axis=mybir.AxisListType.X,
                                    op=mybir.AluOpType.add)
                    nxt = 2 * (g * NP_ + p) + NBAND
                    for d_ in range(2):
                        if nxt + d_ < 2 * B:
                            skew_dma((nxt + d_) // (2 * NP_),
                                     ((nxt + d_) % (2 * NP_)),
                                     (nxt + d_) % NBAND)
                for p in range(NP_):
                    if g == B // 2 - 1 and p == NP_ - 1:
                        nc.sync.dma_start(out=outk[g, :, p, 0],
                                          in_=ob[:, p, 0])
                        nc.sync.dma_start(out=outk[g, :, p, 1],
                                          in_=ob[:, p, 1])
                    else:
                        nc.sync.dma_start(out=outk[g, :, p], in_=ob[:, p])
    nc.compile()
    return nc


def _host_prep(imgs, vers, hors):
    """Build per-core input maps. Returns list of 8 dicts."""
    imgs = np.asarray(imgs, dtype=np.float32)
    vers = np.asarray(vers, dtype=np.float32)
    hors = np.asarray(hors, dtype=np.float32)

    pad_full = np.pad(imgs, ((0, 0), (0, 0), (PAD, PAD), (PAD, PAD)),
                      mode="edge")                       # [B, C, 272, 272]

    hors_r = hors.transpose(0, 2, 3, 1)                  # [B, H, W, K]

    in_maps = []
    for k in range(NCORES):
        w0 = k * WCHUNK
        # pad rows per tile t: padded rows 64t .. 64t+79
        pr = np.empty((B, KT, NT, C, WCHUNK + K - 1), np.float32)
        for t in range(NT):
            pr[:, :, t] = pad_full[:, :, 64 * t:64 * t + KT,
                                   w0:w0 + WCHUNK + K - 1].transpose(0, 2, 1, 3)
        # compact vers for the skew DMA: [g, t, h, u, bi, w]
        vk = np.ascontiguousarray(
            vers[:, :, :, w0:w0 + WCHUNK]                # [B, K, H, Wc]
            .reshape(B // 2, 2, K, NT, T, WCHUNK)        # [g, bi, u, t, h, w]
            .transpose(0, 3, 4, 2, 1, 5))                # [g, t, h, u, bi, w]
        if os.environ.get("SEPCONV_HOSTBAND", "0") == "1":
            # expanded band with zeros: bandk[g, t, r, m, bi, w]
            bnd = np.zeros((B // 2, NT, KT, T, 2, WCHUNK), np.float32)
            for r in range(KT):
                for m in range(T):
                    u = r - m
                    if 0 <= u < K:
                        bnd[:, :, r, m] = vk[:, :, m, u]

        hk = np.ascontiguousarray(
            hors_r[:, :, w0:w0 + WCHUNK, :].reshape(B, NP_, 128, WCHUNK, K)
            .transpose(0, 2, 1, 3, 4))                   # [B, 128, NP_, W, K]
        m = {
            "padk": pr.astype(NPBF16),
            "versk": vk.astype(NPBF16),
            "horsk": hk.astype(NPBF16),
        }
        if os.environ.get("SEPCONV_HOSTBAND", "0") == "1":
            m["bandk"] = bnd.astype(NPBF16)
        in_maps.append(m)
    return in_maps


def _get_dispatch():
    """Build a pre-sharded SPMD dispatcher. Feeding already-sharded device
    arrays avoids jax resharding programs (whose neuronx-cc compile OOMs on
    large inputs)."""
    if "dispatch" in _CACHE:
        return _CACHE["dispatch"]
    import jax
    from jax.experimental.shard_map import shard_map
    from jax.sharding import Mesh, NamedSharding, PartitionSpec
    from concourse import bass2jax

    nc = _CACHE["nc"]
    bass2jax.install_neuronx_cc_hook()
    partition_name = (nc.partition_id_tensor.name
                      if nc.partition_id_tensor else None)
    in_names, out_names, out_avals = [], [], []
    for alloc in nc.m.functions[0].allocations:
        if not isinstance(alloc, mybir.MemoryLocationSet):
            continue
        name = alloc.memorylocations[0].name
        if alloc.kind == "ExternalInput":
            if name != partition_name:
                in_names.append(name)
        elif alloc.kind == "ExternalOutput":
            out_avals.append(jax.core.ShapedArray(tuple(alloc.tensor_shape),
                                                  mybir.dt.np(alloc.dtype)))
            out_names.append(name)
    n_params, n_outs = len(in_names), len(out_names)
    all_in_names = list(in_names) + list(out_names)
    if partition_name is not None:
        all_in_names.append(partition_name)
    all_in_names = tuple(all_in_names)

    def _body(*args):
        operands = list(args)
        if partition_name is not None:
            operands.append(bass2jax.partition_id_tensor())
        outs = bass2jax._bass_exec_p.bind(
            *operands,
            out_avals=tuple(out_avals),
            in_names=all_in_names,
            out_names=tuple(out_names),
            lowering_input_output_aliases=(),
            sim_require_finite=True,
            sim_require_nnan=True,
            nc=nc,
        )
        return tuple(outs)

    devices = jax.devices()[:NCORES]
    mesh = Mesh(np.asarray(devices), ("core",))
    sharding = NamedSharding(mesh, PartitionSpec("core"))
    fn = jax.jit(
        shard_map(_body, mesh=mesh,
                  in_specs=(PartitionSpec("core"),) * (n_params + n_outs),
                  out_specs=(PartitionSpec("core"),) * n_outs,
                  check_rep=False),
        donate_argnums=tuple(range(n_params, n_params + n_outs)),
        keep_unused=True)

    def make_global(shards):
        s0 = shards[0].shape
        arrs = [jax.device_put(shards[c], devices[c]) for c in range(NCORES)]
        return jax.make_array_from_single_device_arrays(
            (NCORES * s0[0], *s0[1:]), sharding, arrs)

    def dispatch(in_maps):
        gin = [make_global([m[name] for m in in_maps]) for name in in_names]
        gzero = [make_global([np.zeros(av.shape, av.dtype)
                              for _ in range(NCORES)])
                 for av in out_avals]
        outs = fn(*gin, *gzero)
        host = [np.asarray(o) for o in outs]
        return [
            {name: host[i].reshape(NCORES, *out_avals[i].shape)[c]
             for i, name in enumerate(out_names)}
            for c in range(NCORES)
        ]

    _CACHE["dispatch"] = dispatch
    return dispatch


class _Res:
    def __init__(self, results, exec_time_ns=None, trace_path=None):
        self.results = results
        self.exec_time_ns = exec_time_ns
        self.instructions_and_trace = ([], trace_path) if trace_path else None


def _sim_time():
    """Cost-model (TimelineSim) per-core time estimate, ns."""
    if "sim_ns" in _CACHE:
        return _CACHE["sim_ns"]
    try:
        from concourse.timeline_sim import TimelineSim
        t = TimelineSim(_CACHE["nc"], trace=False, no_exec=True).simulate()
        _CACHE["sim_ns"] = int(t)
    except Exception:
        _CACHE["sim_ns"] = None
    return _CACHE["sim_ns"]


def _run(in_maps, trace=False):
    if "nc" not in _CACHE:
        _CACHE["nc"] = _build_nc()
    dispatch = _get_dispatch()
    res = _Res(dispatch(in_maps))
    if trace:
        res.exec_time_ns = _sim_time()
    return res


def _assemble(results):
    out = np.empty((B, C, H, W), np.float32)
    for k in range(NCORES):
        ok = np.asarray(results[k]["outk"]).astype(np.float32)
        w0 = k * WCHUNK
        # ok[g, hp, p, bi, w, c] -> out[2g+bi, c, 128p+hp, w0+w]
        out[:, :, :, w0:w0 + WCHUNK] = \
            ok.transpose(0, 3, 5, 2, 1, 4).reshape(B, C, H, WCHUNK)
    return out


def kernel(imgs, vers, hors):
    in_maps = _host_prep(imgs, vers, hors)
    res = _run(in_maps)
    return _assemble(res.results)


def kernel_traced(imgs, vers, hors):
    """Like kernel() but returns (output, results) with a cost-model time."""
    in_maps = _host_prep(imgs, vers, hors)
    res = _run(in_maps, trace=True)
    return _assemble(res.results), res


# revision 35
# speedup vs baseline: 1.2798x; 1.2798x over previous
"""Trainium2 Bass kernel for nn_NewSepConv (per-pixel separable conv, K=17).

out[b,c,h,w] = sum_{u,v} pad[b,c,h+u,w+v] * vers[b,u,h,w] * hors[b,v,h,w]
where pad = replication-pad(imgs, 8).

Decomposition (per batch b, output column w):
  E[h, c, v] = sum_r pad[b, c, r, w+v] * C_w[r, h]          (TensorE matmul)
      with band matrix C_w[r, h] = vers[b, r-h, h, w] (0 <= r-h < 17)
  out[b, c, h, w] = sum_v hors[b, v, h, w] * E[h, c, v]      (DVE mult + reduce)

vs the earlier 32.3us version: the v-reduce is a 5-level tensor_tensor add
tree on DVE (~750ns vs tensor_reduce's ~1068ns per unit), the output ships
bf16 (host converts, halving the out DMA), and two dummy matmuls at t~0.2us
start the PE pstate-ramp clock early so real matmuls run at full clock.

Sharding: 8 cores, each takes a 32-column w-chunk (all batches, all rows).
"""

import numpy as np

import concourse.mybir as mybir
import concourse.tile as tile
from concourse import bacc
from concourse.bass_utils import run_bass_kernel_spmd

F32 = mybir.dt.float32
BF16 = mybir.dt.bfloat16
NPBF16 = mybir.dt.np(mybir.dt.bfloat16)

B, C, H, W = 4, 3, 256, 256
K = 17
PAD = 8
NCORES = 8
WCHUNK = W // NCORES  # 32

T = 64                # h-tile size
NT = H // T           # 4 h-tiles
KT = T + K - 1        # 80 r-rows per tile
NP_ = 2               # psum pairs (2 h-tiles each -> 128 partitions)
WG = 16               # w-columns per psum tile (2-bank padded slots)
NWG = WCHUNK // WG    # 2

_CACHE = {}


def _build_nc():
    nc = bacc.Bacc("TRN2", target_bir_lowering=False, debug=False)
    padk = nc.dram_tensor("padk", [B, KT, NT, C, WCHUNK + K - 1], BF16,
                          kind="ExternalInput").ap()
    bandk = nc.dram_tensor("bandk", [B, KT, NT, WCHUNK, T], BF16,
                           kind="ExternalInput").ap()
    horsk = nc.dram_tensor("horsk", [B, 128, NP_, WCHUNK, K], BF16,
                           kind="ExternalInput").ap()
    outk = nc.dram_tensor("outk", [B // 2, 128, 2, NP_, WCHUNK, C], BF16,
                          kind="ExternalOutput").ap()

    with tile.TileContext(nc) as tc:
        with tc.tile_pool(name="pads", bufs=2) as pad_pool, \
             tc.tile_pool(name="bands", bufs=2) as band_pool, \
             tc.tile_pool(name="hors", bufs=2) as hors_pool, \
             tc.tile_pool(name="mtmp", bufs=12) as m_pool, \
             tc.tile_pool(name="tree", bufs=12) as t_pool, \
             tc.tile_pool(name="obuf", bufs=2) as o_pool, \
             tc.tile_pool(name="psum", bufs=4, space="PSUM") as psum_pool:
            def _emit_reduce(mt, ob, bi, p, wg):
                # 5-level tensor_tensor add tree over v=17 (DVE, ~750ns vs
                # tensor_reduce's ~1068ns; bf16 partials are in tolerance)
                a = t_pool.tile([128, WG, C, 8], BF16, tag="ta", name="ta")
                nc.vector.tensor_tensor(out=a[:], in0=mt[:, :, :, 0:8],
                                        in1=mt[:, :, :, 8:16],
                                        op=mybir.AluOpType.add)
                b2 = t_pool.tile([128, WG, C, 4], BF16, tag="tb", name="tb")
                nc.vector.tensor_tensor(out=b2[:], in0=a[:, :, :, 0:4],
                                        in1=a[:, :, :, 4:8],
                                        op=mybir.AluOpType.add)
                c2 = t_pool.tile([128, WG, C, 2], BF16, tag="tc", name="tc")
                nc.vector.tensor_tensor(out=c2[:], in0=b2[:, :, :, 0:2],
                                        in1=b2[:, :, :, 2:4],
                                        op=mybir.AluOpType.add)
                d = t_pool.tile([128, WG, C], BF16, tag="td", name="td")
                nc.vector.tensor_tensor(out=d[:], in0=c2[:, :, :, 0],
                                        in1=c2[:, :, :, 1],
                                        op=mybir.AluOpType.add)
                nc.vector.tensor_tensor(
                    out=ob[:, bi, p, wg * WG:(wg + 1) * WG, :],
                    in0=d[:], in1=mt[:, :, :, 16],
                    op=mybir.AluOpType.add)

            # PE pstate warmup: the cost model ramps the PE clock from
            # 0.65GHz to 2.4GHz over ~3us from the first matmul; two dummy
            # matmuls right at the start make every real matmul full-speed.
            warm = m_pool.tile([128, 64], BF16, tag="warm", name="warm")
            nc.vector._memset_packed(warm[:].bitcast(mybir.dt.uint32), 0)
            psw = psum_pool.tile([128, WG, 64], F32, tag="ps", name="psw")
            for _ in range(2):
                nc.tensor.matmul(out=psw[0:2, 0, 0:64], lhsT=warm[:, 0:2],
                                 rhs=warm[:], start=True, stop=True)
            for g in range(B // 2):
                pad_all = pad_pool.tile([KT, 2, NT, C, WCHUNK + K - 1], BF16,
                                        tag="pad")
                band_all = band_pool.tile([KT, 2, NT, WCHUNK, T], BF16,
                                          tag="band")
                hors_all = hors_pool.tile([128, 2, NP_, WCHUNK, K], BF16,
                                          tag="hors")
                if g == 0:
                    # fine-grained ramp-in: unblock the first matmuls fast
                    nc.sync.dma_start(out=band_all[:, 0, 0],
                                      in_=bandk[0, :, 0])
                    nc.sync.dma_start(out=band_all[:, 0, 1],
                                      in_=bandk[0, :, 1])
                    nc.sync.dma_start(out=pad_all[:, 0], in_=padk[0])
                    nc.sync.dma_start(out=hors_all[:, 0], in_=horsk[0])
                    for t in range(2, NT):
                        nc.sync.dma_start(out=band_all[:, 0, t],
                                          in_=bandk[0, :, t])
                    nc.sync.dma_start(out=band_all[:, 1], in_=bandk[1])
                    nc.sync.dma_start(out=pad_all[:, 1], in_=padk[1])
                    nc.sync.dma_start(out=hors_all[:, 1], in_=horsk[1])
                else:
                    nc.sync.dma_start(out=band_all[:, 0],
                                      in_=bandk[2 * g])
                    nc.sync.dma_start(
                        out=pad_all[:],
                        in_=padk[2 * g:2 * g + 2].transpose([1, 0, 2, 3, 4]))
                    nc.sync.dma_start(out=hors_all[:, 0],
                                      in_=horsk[2 * g])
                    nc.sync.dma_start(out=band_all[:, 1],
                                      in_=bandk[2 * g + 1])
                    nc.sync.dma_start(out=hors_all[:, 1],
                                      in_=horsk[2 * g + 1])
                ob = o_pool.tile([128, 2, NP_, WCHUNK, C], BF16, tag="ob")
                for bi in range(2):
                    for p in range(NP_):
                        for wg in range(NWG):
                            ps = psum_pool.tile([128, WG, 64], F32,
                                                tag="ps", name="ps")
                            for wl8 in range(WG):
                                wl = wg * WG + wl8
                                for half in range(2):
                                    t = 2 * p + half
                                    nc.tensor.matmul(
                                        out=ps[64 * half:64 * half + 64,
                                               wl8, 0:C * K],
                                        lhsT=band_all[:, bi, t, wl, :],
                                        rhs=pad_all[:, bi, t, :, wl:wl + K],
                                        start=True, stop=True,
                                    )
                            et = m_pool.tile([128, WG, C, K], BF16, tag="et")
                            ps_v = ps[:, :, 0:C * K].rearrange(
                                "p w (c v) -> p w c v", c=C)
                            nc.scalar.copy(out=et[:], in_=ps_v)
                            mt = m_pool.tile([128, WG, C, K], BF16, tag="mt")
                            hslice = hors_all[:, bi, p,
                                              wg * WG:(wg + 1) * WG, :]
                            hb = hslice.unsqueeze(2).broadcast_to(
                                [128, WG, C, K])
                            eng = nc.vector if wg % 2 == 0 else nc.gpsimd
                            eng.tensor_tensor(out=mt[:], in0=et[:], in1=hb,
                                              op=mybir.AluOpType.mult)
                            _emit_reduce(mt, ob, bi, p, wg)
                if g == B // 2 - 1:
                    nc.sync.dma_start(out=outk[g, :, 0], in_=ob[:, 0])
                    nc.sync.dma_start(out=outk[g, :, 1], in_=ob[:, 1])
                else:
                    nc.sync.dma_start(out=outk[g], in_=ob[:])
    nc.compile()
    return nc


def _host_prep(imgs, vers, hors):
    """Build per-core input maps. Returns list of 8 dicts."""
    imgs = np.asarray(imgs, dtype=np.float32)
    vers = np.asarray(vers, dtype=np.float32)
    hors = np.asarray(hors, dtype=np.float32)

    pad_full = np.pad(imgs, ((0, 0), (0, 0), (PAD, PAD), (PAD, PAD)),
                      mode="edge")                       # [B, C, 272, 272]

    # band_full[b, t, r, m, w] = vers[b, r-m, 64t+m, w]  (zeros outside band)
    r_idx = np.arange(KT)[:, None]
    m_idx = np.arange(T)[None, :]
    u = r_idx - m_idx
    u_ok = ((u >= 0) & (u < K)).astype(np.float32)       # [KT, T]
    uc = np.clip(u, 0, K - 1)
    band_ts = []
    for t in range(NT):
        h_grid = np.broadcast_to(T * t + m_idx, (KT, T))
        a = vers[:, uc, h_grid, :]                       # [B, KT, T, W]
        a *= u_ok[None, :, :, None]
        band_ts.append(a)
    band_full = np.stack(band_ts, axis=1)                # [B, NT, KT, T, W]

    hors_r = hors.transpose(0, 2, 3, 1)                  # [B, H, W, K]

    in_maps = []
    for k in range(NCORES):
        w0 = k * WCHUNK
        # pad rows per tile t: padded rows 64t .. 64t+79
        pr = np.empty((B, KT, NT, C, WCHUNK + K - 1), np.float32)
        for t in range(NT):
            pr[:, :, t] = pad_full[:, :, 64 * t:64 * t + KT,
                                   w0:w0 + WCHUNK + K - 1].transpose(0, 2, 1, 3)
        bandk = np.ascontiguousarray(                    # [B, KT, NT, W, T]
            band_full[:, :, :, :, w0:w0 + WCHUNK].transpose(0, 2, 1, 4, 3))
        hk = np.ascontiguousarray(
            hors_r[:, :, w0:w0 + WCHUNK, :].reshape(B, NP_, 128, WCHUNK, K)
            .transpose(0, 2, 1, 3, 4))                   # [B, 128, NP_, W, K]
        in_maps.append({
            "padk": pr.astype(NPBF16),
            "bandk": bandk.astype(NPBF16),
            "horsk": hk.astype(NPBF16),
        })
    return in_maps


def _get_dispatch():
    """Build a pre-sharded SPMD dispatcher. Feeding already-sharded device
    arrays avoids jax resharding programs (whose neuronx-cc compile OOMs on
    large inputs)."""
    if "dispatch" in _CACHE:
        return _CACHE["dispatch"]
    import jax
    from jax.experimental.shard_map import shard_map
    from jax.sharding import Mesh, NamedSharding, PartitionSpec
    from concourse import bass2jax

    nc = _CACHE["nc"]
    bass2jax.install_neuronx_cc_hook()
    partition_name = (nc.partition_id_tensor.name
                      if nc.partition_id_tensor else None)
    in_names, out_names, out_avals = [], [], []
    for alloc in nc.m.functions[0].allocations:
        if not isinstance(alloc, mybir.MemoryLocationSet):
            continue
        name = alloc.memorylocations[0].name
        if alloc.kind == "ExternalInput":
            if name != partition_name:
                in_names.append(name)
        elif alloc.kind == "ExternalOutput":
            out_avals.append(jax.core.ShapedArray(tuple(alloc.tensor_shape),
                                                  mybir.dt.np(alloc.dtype)))
            out_names.append(name)
    n_params, n_outs = len(in_names), len(out_names)
    all_in_names = list(in_names) + list(out_names)
    if partition_name is not None:
        all_in_names.append(partition_name)
    all_in_names = tuple(all_in_names)

    def _body(*args):
        operands = list(args)
        if partition_name is not None:
            operands.append(bass2jax.partition_id_tensor())
        outs = bass2jax._bass_exec_p.bind(
            *operands,
            out_avals=tuple(out_avals),
            in_names=all_in_names,
            out_names=tuple(out_names),
            lowering_input_output_aliases=(),
            sim_require_finite=True,
            sim_require_nnan=True,
            nc=nc,
        )
        return tuple(outs)

    devices = jax.devices()[:NCORES]
    mesh = Mesh(np.asarray(devices), ("core",))
    sharding = NamedSharding(mesh, PartitionSpec("core"))
    fn = jax.jit(
        shard_map(_body, mesh=mesh,
                  in_specs=(PartitionSpec("core"),) * (n_params + n_outs),
                  out_specs=(PartitionSpec("core"),) * n_outs,
                  check_rep=False),
        donate_argnums=tuple(range(n_params, n_params + n_outs)),
        keep_unused=True)

    def make_global(shards):
        s0 = shards[0].shape
        arrs = [jax.device_put(shards[c], devices[c]) for c in range(NCORES)]
        return jax.make_array_from_single_device_arrays(
            (NCORES * s0[0], *s0[1:]), sharding, arrs)

    def dispatch(in_maps):
        gin = [make_global([m[name] for m in in_maps]) for name in in_names]
        gzero = [make_global([np.zeros(av.shape, av.dtype)
                              for _ in range(NCORES)])
                 for av in out_avals]
        outs = fn(*gin, *gzero)
        host = [np.asarray(o) for o in outs]
        return [
            {name: host[i].reshape(NCORES, *out_avals[i].shape)[c]
             for i, name in enumerate(out_names)}
            for c in range(NCORES)
        ]

    _CACHE["dispatch"] = dispatch
    return dispatch


class _Res:
    def __init__(self, results, exec_time_ns=None, trace_path=None):
        self.results = results
        self.exec_time_ns = exec_time_ns
        self.instructions_and_trace = ([], trace_path) if trace_path else None


def _sim_time():
    """Cost-model (TimelineSim) per-core time estimate, ns."""
    if "sim_ns" in _CACHE:
        return _CACHE["sim_ns"]
    try:
        from concourse.timeline_sim import TimelineSim
        t = TimelineSim(_CACHE["nc"], trace=False, no_exec=True).simulate()
        _CACHE["sim_ns"] = int(t)
    except Exception:
        _CACHE["sim_ns"] = None
    return _CACHE["sim_ns"]


def _run(in_maps, trace=False):
    if "nc" not in _CACHE:
        _CACHE["nc"] = _build_nc()
    dispatch = _get_dispatch()
    res = _Res(dispatch(in_maps))
    if trace:
        res.exec_time_ns = _sim_time()
    return res


def _assemble(results):
    out = np.empty((B, C, H, W), np.float32)
    for k in range(NCORES):
        ok = np.asarray(results[k]["outk"]).astype(np.float32)
        w0 = k * WCHUNK
        # ok[g, hp, bi, p, w, c] -> out[2g+bi, c, 128p+hp, w0+w]
        out[:, :, :, w0:w0 + WCHUNK] = \
            ok.transpose(0, 2, 5, 3, 1, 4).reshape(B, C, H, WCHUNK)
    return out


def kernel(imgs, vers, hors):
    in_maps = _host_prep(imgs, vers, hors)
    res = _run(in_maps)
    return _assemble(res.results)


def kernel_traced(imgs, vers, hors):
    """Like kernel() but returns (output, results) with a cost-model time."""
    in_maps = _host_prep(imgs, vers, hors)
    res = _run(in_maps, trace=True)
    return _assemble(res.results), res
